# revision 4
# baseline (speedup 1.0000x reference)
"""Attention1D Trainium2 Bass kernel, sharded over 8 NeuronCores.

Reference computation (per batch b, C=512 channels, T=2048, H=8 heads, d=64):
    qkv = qkv_w @ x + qkv_b            # [3C, T]
    q, k, v = split(qkv)               # each [C, T], viewed as H heads of d=64
    attn = softmax((q_h . k_h) * C**-0.5, over s)
    out_h = attn @ v_h
    out = proj_w @ concat(out_h) + proj_b
    result = x + out

Sharding: 8 cores = 4 batches x 2 head-groups (4 heads each).  Each core
computes its group's partial projection output y_g = proj_w[:, g] @ attn_g;
the host combines: out[b] = x[b] + (proj_b + proj_w @ v_bias) + y_0 + y_1.
(The v bias commutes through softmax-weighted averaging because the
probabilities sum to 1, so it folds into an effective projection bias.)

On-core layout: everything is kept [channel-like, t] with t in the free dim.
Scores are computed transposed (S^T[s, t]) so the exp output E feeds the
A.V matmul directly as the moving operand with K = s.  Softmax denominators
come from an extra ones-weight matmul (column sums of E).  K=64 score matmuls
for the two heads of a pair are row-packed at partitions 0/64 so they run
concurrently in the PE array.  All matmuls use float32r (full-rate fp32).
"""

import sys

if "/opt/trn_rl_repo" not in sys.path:
    sys.path.insert(0, "/opt/trn_rl_repo")

import numpy as np

B, C, T, H = 4, 512, 2048, 8
D = C // H  # 64 head dim
G = 2  # head groups (cores per batch)
CG = C // G  # 256 channels per group
SCALE = float(C) ** -0.5
N_CORES = 8

_CACHE: dict = {}


def _build_nc():
    import concourse.tile as tile
    from concourse import bacc, mybir

    f32 = mybir.dt.float32
    f32r = mybir.dt.float32r
    Exp = mybir.ActivationFunctionType.Exp

    nc = bacc.Bacc(
        "TRN2", target_bir_lowering=False, debug=False, num_devices=N_CORES
    )
    x = nc.dram_tensor("x", [C, T], f32, kind="ExternalInput").ap()
    wqk = nc.dram_tensor("wqkT", [C, 2 * CG], f32, kind="ExternalInput").ap()
    wv = nc.dram_tensor("wvT", [C, CG], f32, kind="ExternalInput").ap()
    wp = nc.dram_tensor("wpT", [CG, C], f32, kind="ExternalInput").ap()
    bqk = nc.dram_tensor("bqk", [2 * CG, 1], f32, kind="ExternalInput").ap()
    y = nc.dram_tensor("y", [C, T], f32, kind="ExternalOutput").ap()

    NQ = T // 512  # 4 moving-dim chunks of 512
    NCT = T // 128  # 16 contraction chunks of 128 (s dim)

    with tile.TileContext(nc) as tc:
        with tc.tile_pool(name="persist", bufs=1) as pp:
            x_sb = pp.tile([128, 4, T], f32r)
            wqk_sb = pp.tile([128, 4, 2 * CG], f32r)
            wv_sb = pp.tile([128, 4, CG], f32r)
            wp_sb = pp.tile([128, 2, C], f32r)
            bqk_sb = pp.tile([128, 4], f32)
            ones_sb = pp.tile([128, 128], f32r)
            qk_sb = pp.tile([128, 4, T], f32r)
            # av lhsT slots per (chunk, pair): [vt_even | 0 | 0 | vt_odd]
            vt_sb = pp.tile([128, NCT, 2, 4, 64], f32r)
            attn_sb = pp.tile([128, 2, T], f32r)

            # gpsimd DMAs cast f32 -> f32r (rounding) on the fly.
            nc.gpsimd.dma_start(
                out=x_sb, in_=x.rearrange("(kt p) t -> p kt t", p=128)
            )
            nc.gpsimd.dma_start(
                out=wqk_sb, in_=wqk.rearrange("(kt p) m -> p kt m", p=128)
            )
            nc.gpsimd.dma_start(
                out=wv_sb, in_=wv.rearrange("(kt p) m -> p kt m", p=128)
            )
            nc.gpsimd.dma_start(
                out=wp_sb, in_=wp.rearrange("(kt p) m -> p kt m", p=128)
            )
            nc.sync.dma_start(
                out=bqk_sb, in_=bqk.rearrange("(mt p) one -> p (mt one)", p=128)
            )
            nc.vector.memset(ones_sb[:].bitcast(f32), 1.0)
            nc.vector.memset(vt_sb[:].bitcast(f32), 0.0)

            # ---- Phase A: q/k projections (with bias) and v^T ----
            with tc.tile_pool(name="psA", bufs=1, space="PSUM") as psA:
                # qk: out rows mt: 0 = q heads 0-1, 1 = q heads 2-3,
                #               2 = k heads 0-1, 3 = k heads 2-3
                for mt in range(4):
                    for nq in range(NQ):
                        ps = psA.tile([128, 512], f32, tag="qk", bufs=3)
                        for kt in range(4):
                            nc.tensor.matmul(
                                out=ps,
                                lhsT=wqk_sb[:, kt, mt * 128 : (mt + 1) * 128],
                                rhs=x_sb[:, kt, nq * 512 : (nq + 1) * 512],
                                start=(kt == 0),
                                stop=(kt == 3),
                            )
                        nc.vector.tensor_scalar_add(
                            out=qk_sb[:, mt, nq * 512 : (nq + 1) * 512],
                            in0=ps,
                            scalar1=bqk_sb[:, mt : mt + 1],
                        )
                # v^T: [t, c'] tiles; no bias (folded into host-side proj bias)
                for ct in range(NCT):
                    ps = psA.tile([128, CG], f32, tag="vt", bufs=3)
                    for kt in range(4):
                        nc.tensor.matmul(
                            out=ps,
                            lhsT=x_sb[:, kt, ct * 128 : (ct + 1) * 128],
                            rhs=wv_sb[:, kt, :],
                            start=(kt == 0),
                            stop=(kt == 3),
                        )
                    ps_v = ps.rearrange("p (pr parity d) -> p pr parity d", pr=2, parity=2)
                    nc.vector.tensor_copy(out=vt_sb[:, ct, :, 0, :], in_=ps_v[:, :, 0, :])
                    nc.vector.tensor_copy(out=vt_sb[:, ct, :, 3, :], in_=ps_v[:, :, 1, :])

            # ---- Phase B: attention per head-pair p, per t-chunk tq ----
            with (
                tc.tile_pool(name="psB", bufs=1, space="PSUM") as psB,
                tc.tile_pool(name="epool", bufs=3) as epool,
                tc.tile_pool(name="rpool", bufs=4) as rpool,
            ):
                for p in range(2):
                    q_t = qk_sb[:, p, :]
                    k_t = qk_sb[:, 2 + p, :]
                    for tq in range(NQ):
                        ts = slice(tq * 512, (tq + 1) * 512)
                        # numerators (even head -> rows 0:64 of av_a, odd
                        # head -> rows 64:128 of av_b) and denominators;
                        # four separate PSUM banks so each bank holds one
                        # accumulation group.
                        av_a = psB.tile([128, 512], f32, tag="ava", bufs=1)
                        av_b = psB.tile([128, 512], f32, tag="avb", bufs=1)
                        zb_a = psB.tile([128, 512], f32, tag="zba", bufs=1)
                        zb_b = psB.tile([128, 512], f32, tag="zbb", bufs=1)
                        for ct in range(NCT):
                            cs = slice(ct * 128, (ct + 1) * 128)
                            sc = psB.tile([128, 1024], f32, tag="sc", bufs=2)
                            # scores S^T[s, t] for the two heads, row-packed
                            # K=64 at partitions 0 / 64.
                            nc.tensor.matmul(
                                out=sc[:, 0:512],
                                lhsT=k_t[0:64, cs],
                                rhs=q_t[0:64, ts],
                                start=True,
                                stop=True,
                            )
                            nc.tensor.matmul(
                                out=sc[:, 512:1024],
                                lhsT=k_t[64:128, cs],
                                rhs=q_t[64:128, ts],
                                start=True,
                                stop=True,
                            )
                            e_t = epool.tile([128, 1024], f32r, tag="e")
                            nc.scalar.activation(
                                out=e_t, in_=sc[:], func=Exp, scale=SCALE
                            )
                            nc.tensor.matmul(
                                out=av_a[:],
                                lhsT=vt_sb[:, ct, p, 0:2, :],
                                rhs=e_t[:, 0:512],
                                start=(ct == 0),
                                stop=(ct == NCT - 1),
                            )
                            nc.tensor.matmul(
                                out=av_b[:],
                                lhsT=vt_sb[:, ct, p, 2:4, :],
                                rhs=e_t[:, 512:1024],
                                start=(ct == 0),
                                stop=(ct == NCT - 1),
                            )
                            nc.tensor.matmul(
                                out=zb_a[:],
                                lhsT=ones_sb[:],
                                rhs=e_t[:, 0:512],
                                start=(ct == 0),
                                stop=(ct == NCT - 1),
                            )
                            nc.tensor.matmul(
                                out=zb_b[:],
                                lhsT=ones_sb[:],
                                rhs=e_t[:, 512:1024],
                                start=(ct == 0),
                                stop=(ct == NCT - 1),
                            )
                        rc_a = rpool.tile([128, 512], f32, tag="rc")
                        nc.vector.reciprocal(out=rc_a[0:64, :], in_=zb_a[0:64, :])
                        nc.vector.tensor_mul(
                            out=attn_sb[0:64, p, ts],
                            in0=av_a[0:64, :],
                            in1=rc_a[0:64, :],
                        )
                        rc_b = rpool.tile([128, 512], f32, tag="rc")
                        nc.vector.reciprocal(
                            out=rc_b[64:128, :], in_=zb_b[64:128, :]
                        )
                        nc.vector.tensor_mul(
                            out=attn_sb[64:128, p, ts],
                            in0=av_b[64:128, :],
                            in1=rc_b[64:128, :],
                        )

            # ---- Phase C: partial projection ----
            with (
                tc.tile_pool(name="psC", bufs=1, space="PSUM") as psC,
                tc.tile_pool(name="ypool", bufs=2) as ypool,
            ):
                y_r = y.rearrange("(mt p) t -> p mt t", p=128)
                for mt in range(4):
                    y_t = ypool.tile([128, T], f32, tag="y")
                    for nq in range(NQ):
                        ps = psC.tile([128, 512], f32, tag="pj", bufs=4)
                        for kt in range(2):
                            nc.tensor.matmul(
                                out=ps,
                                lhsT=wp_sb[:, kt, mt * 128 : (mt + 1) * 128],
                                rhs=attn_sb[:, kt, nq * 512 : (nq + 1) * 512],
                                start=(kt == 0),
                                stop=(kt == 1),
                            )
                        nc.vector.tensor_copy(
                            out=y_t[:, nq * 512 : (nq + 1) * 512], in_=ps
                        )
                    nc.sync.dma_start(out=y_r[:, mt, :], in_=y_t)

    nc.compile()
    return nc


def _get_runner():
    """Build (once) a cached jitted 8-core SPMD executor for the kernel."""
    if "runner" in _CACHE:
        return _CACHE["runner"]

    import jax
    import numpy as _np
    from jax.sharding import Mesh, PartitionSpec
    from jax.experimental.shard_map import shard_map

    from concourse import bass2jax, mybir

    nc = _build_nc()
    bass2jax.install_neuronx_cc_hook()

    partition_name = (
        nc.partition_id_tensor.name if nc.partition_id_tensor else None
    )
    in_names: list[str] = []
    out_names: list[str] = []
    out_avals = []
    zero_outs: list[_np.ndarray] = []
    for alloc in nc.m.functions[0].allocations:
        if not isinstance(alloc, mybir.MemoryLocationSet):
            continue
        name = alloc.memorylocations[0].name
        if alloc.kind == "ExternalInput":
            if name != partition_name:
                in_names.append(name)
        elif alloc.kind == "ExternalOutput":
            shape = tuple(alloc.tensor_shape)
            dtype = mybir.dt.np(alloc.dtype)
            out_names.append(name)
            out_avals.append(jax.core.ShapedArray(shape, dtype))
            zero_outs.append(_np.zeros(shape, dtype))
    n_params = len(in_names)
    n_outs = len(out_avals)
    in_names_all = in_names + out_names
    if partition_name is not None:
        in_names_all.append(partition_name)

    donate = tuple(range(n_params, n_params + n_outs))

    def _body(*args):
        operands = list(args)
        if partition_name is not None:
            operands.append(bass2jax.partition_id_tensor())
        outs = bass2jax._bass_exec_p.bind(
            *operands,
            out_avals=tuple(out_avals),
            in_names=tuple(in_names_all),
            out_names=tuple(out_names),
            lowering_input_output_aliases=(),
            sim_require_finite=True,
            sim_require_nnan=True,
            nc=nc,
        )
        return tuple(outs)

    devices = jax.devices()[:N_CORES]
    mesh = Mesh(np.asarray(devices), ("core",))
    in_specs = (PartitionSpec("core"),) * (n_params + n_outs)
    out_specs = (PartitionSpec("core"),) * n_outs
    sharded = jax.jit(
        shard_map(
            _body, mesh=mesh, in_specs=in_specs, out_specs=out_specs,
            check_rep=False,
        ),
        donate_argnums=donate,
        keep_unused=True,
    )

    runner = {
        "fn": sharded,
        "in_names": in_names,
        "out_names": out_names,
        "zero_outs": zero_outs,
    }
    _CACHE["runner"] = runner
    return runner


def _prepare_in_maps(x, qkv_w, qkv_b, proj_w, proj_b):
    """Full inputs -> per-core input dicts (batch x head-group sharding)."""
    in_maps = []
    for c in range(N_CORES):
        b, g = divmod(c, G)
        qs = slice(g * CG, (g + 1) * CG)
        wq = qkv_w[qs, :]
        wk = qkv_w[C + g * CG : C + (g + 1) * CG, :]
        wv = qkv_w[2 * C + g * CG : 2 * C + (g + 1) * CG, :]
        wqkT = np.ascontiguousarray(np.concatenate([wq, wk], axis=0).T)
        wvT = np.ascontiguousarray(wv.T)
        wpT = np.ascontiguousarray(proj_w[:, qs].T)
        bqk = np.ascontiguousarray(
            np.concatenate([qkv_b[qs], qkv_b[C + g * CG : C + (g + 1) * CG]])
        ).reshape(2 * CG, 1)
        in_maps.append(
            {
                "x": np.ascontiguousarray(x[b]),
                "wqkT": wqkT,
                "wvT": wvT,
                "wpT": wpT,
                "bqk": bqk.astype(np.float32),
            }
        )
    return in_maps


def _run_in_maps(in_maps):
    """Run the SPMD kernel, return list of per-core output dicts."""
    import jax

    r = _get_runner()
    per_core = [
        [np.asarray(m[name]) for name in r["in_names"]] for m in in_maps
    ]
    concat_in = [
        np.concatenate([per_core[c][i] for c in range(N_CORES)], axis=0)
        for i in range(len(r["in_names"]))
    ]
    concat_zero = [
        np.concatenate([z] * N_CORES, axis=0) for z in r["zero_outs"]
    ]
    outs = r["fn"](*concat_in, *concat_zero)
    outs = [np.asarray(o) for o in outs]
    results = []
    for c in range(N_CORES):
        d = {}
        for i, name in enumerate(r["out_names"]):
            per_len = outs[i].shape[0] // N_CORES
            d[name] = outs[i][c * per_len : (c + 1) * per_len]
        results.append(d)
    return results


def kernel(x, qkv_w, qkv_b, proj_w, proj_b):
    x = np.asarray(x, dtype=np.float32)
    qkv_w = np.asarray(qkv_w, dtype=np.float32)
    qkv_b = np.asarray(qkv_b, dtype=np.float32)
    proj_w = np.asarray(proj_w, dtype=np.float32)
    proj_b = np.asarray(proj_b, dtype=np.float32)

    in_maps = _prepare_in_maps(x, qkv_w, qkv_b, proj_w, proj_b)
    results = _run_in_maps(in_maps)

    # host combine: residual + effective projection bias + the two
    # head-group partials per batch.
    bp_eff = proj_b + proj_w @ qkv_b[2 * C : 3 * C]
    out = np.empty((B, C, T), dtype=np.float32)
    for b in range(B):
        out[b] = (
            x[b]
            + bp_eff[:, None]
            + results[G * b]["y"]
            + results[G * b + 1]["y"]
        )
    return out
